# revision 28
# baseline (speedup 1.0000x reference)
"""Trainium2 Bass kernel for nn_DownsamplePoly (resample_poly up=5/down=64,
269-tap polyphase filter, x:[16,1280000,4] fp32 -> y:[16,100000,4] fp32).

Strategy
--------
Math: y[n, c] = sum_t coef(n, t) * x[t, c], coef(n, t) = h[(n+11)*64 - 5t]
(zero outside [0,1345)). Tiling outputs in blocks of M=120 (M(n) advances
exactly 1536 samples = 12 aligned 128-chunks per 120 outputs), each block
needs 14 aligned 128-sample input chunks, and the 14 banded weight
matrices W_j[k, m] = h[64m + 1344 - 640j - 5k] are INDEPENDENT of the
block index. The resampler is a pump of PSUM-accumulated
[128k x 128m] @ [128k x 512] matmuls with 14 fixed fp16 weight matrices.

x is quantized on host to fp8-e4m3 with FIRST-ORDER NOISE SHAPING (error
feedback along t): the 1345-tap filter is a narrow lowpass (cutoff pi/64),
so the shaped (high-passed) quantization noise is rejected by ~40 dB;
measured end-to-end error ~4e-3 relative vs the f32 reference, while
halving HBM traffic vs fp16 (10.4 MB/core -> DMA fully hidden under
compute). Weights stay fp16 (mixed fp8 x fp16 matmul, fp32 PSUM
accumulate). 8 cores split the batch dim (2 batches/core).

Schedule: supertiles batch 8(b,c) x jp J-tiles as the moving free dim
(max 512 cols); sizes ramp 16,16,32,48,64... so real matmuls start as
soon as the first small slab lands. 24 warm-up matmuls on garbage data
bridge the ~7us engine-boot prologue and ~4us first-DMA latency so the
PE's activity monitor (HAM) reaches full clock (2.4 GHz) on every core
before the pump starts and never re-throttles. Steady state is
tensor-engine bound: 14 x 512 cycles per supertile (3.02us measured,
2.99us theoretical), 93.4k PE cycles/core total. HW exec ~60.5us
(baseline 89.9us).
"""

import os
from contextlib import ExitStack

import numpy as np
import ml_dtypes
from numpy.lib.stride_tricks import as_strided

# ---- geometry (hardcoded for this problem) ----
B, T, C = 16, 1_280_000, 4
N_OUT = 100_000
SU, DU = 50, 640          # -> up=5, down=64
MT = 120                  # outputs per J-tile (psum partition dim, padded 128)
KCH = 14                  # chunk-matmuls per J-tile
AQ = 12                   # chunk advance per J-tile
JP = 64                   # max J-tiles per supertile
# ramped supertile sizes: small ones first so real matmuls start as soon as
# the first small slab lands and the PE stays continuously busy (HAM warm)
JPS = [16, 16, 32, 48] + [64] * 11 + [18]
NS = len(JPS)             # 16 supertiles, 834 J-tiles total
CUMJ = [0]
for _jp in JPS:
    CUMJ.append(CUMJ[-1] + _jp)
JTOT = CUMJ[-1]
Q8 = JP + 1               # 65: max q8 range within a slab
SLAB_Q = AQ * Q8          # 780 chunks per max slab
PAD_L = 128               # x_pad[b, i] = x[b, i-128]
BPC = B // 8              # batches per core = 2
NBC = BPC * C             # 8 (b,c) pairs per core
QPAD = 10020
F8NP = ml_dtypes.float8_e4m3

_NC_CACHE = {}

# ---------------- filter / weights ----------------


def _build_filter():
    # replicates reference._make_filter(640, 50, T) without reading files
    from math import gcd

    g = gcd(SU, DU)
    up, down = SU // g, DU // g  # 5, 64
    max_rate = max(up, down)
    half_len = 10 * max_rate
    numtaps = 2 * half_len + 1
    m = np.arange(numtaps) - (numtaps - 1) / 2.0
    cutoff = 1.0 / max_rate
    h = cutoff * np.sinc(cutoff * m)
    h *= np.kaiser(numtaps, 5.0)
    h /= h.sum()
    h = h * up
    n_pre_pad = down - half_len % down
    n_out = T * up // down + bool((T * up) % down)
    n_pre_remove = (half_len + n_pre_pad) // down

    def _output_len(len_h, in_len):
        return ((in_len - 1) * up + len_h - 1) // down + 1

    n_post_pad = 0
    while _output_len(numtaps + n_pre_pad + n_post_pad, T) < n_out + n_pre_remove:
        n_post_pad += 1
    return np.concatenate(
        [np.zeros(n_pre_pad), h, np.zeros(n_post_pad)]
    ).astype(np.float32)


def build_weights(h):
    """W[j, k, m] = h_ext[64m + 1344 - 640j - 5k], the 14 banded matrices."""
    h_ext = np.zeros(1345 + 16384, dtype=np.float32)
    h_ext[: h.shape[0]] = h
    j = np.arange(KCH)[:, None, None]
    k = np.arange(128)[None, :, None]
    m = np.arange(MT)[None, None, :]
    idx = 64 * m + 1344 - 640 * j - 5 * k
    valid = (idx >= 0) & (idx <= 1344)
    return np.where(valid, h_ext[np.clip(idx, 0, 1344)], 0.0).astype(np.float32)


# ---------------- noise-shaped fp8 quantizer ----------------

FRAG = 1024
WIN = 64
HWIN = WIN // 2
_LUTS = {}


def _luts():
    if not _LUTS:
        allu16 = np.arange(65536, dtype=np.uint16)
        f16 = allu16.view(np.float16)
        with np.errstate(invalid="ignore", over="ignore"):
            q8 = f16.astype(np.float32).astype(F8NP)
        _LUTS["b"] = q8.view(np.uint8).copy()   # f16 bits -> e4m3 byte
        _LUTS["f"] = q8.astype(np.float32)      # f16 bits -> e4m3 value
    return _LUTS["b"], _LUTS["f"]


def shaped_quant(xs):
    """xs: [S, T] float32, T multiple of FRAG. Returns e4m3 bytes [S, T] u8.

    First-order error feedback q[t] = Q(x[t] + e[t-1]) shapes quantization
    noise as (1-z^-1)e. Vectorized across (series, fragment) rows; each
    fragment's feedback resets just after a min-|x| sample chosen in a
    +/-32 window at the fragment edge, so the seam impulse (the carried
    e-state) is bounded by the rounding ulp of a near-zero sample.
    """
    lutb, lutf = _luts()
    S, Tx = xs.shape
    nf = Tx // FRAG
    L = FRAG + WIN
    xp = np.concatenate([np.zeros((S, HWIN), np.float32), xs,
                         np.zeros((S, HWIN), np.float32)], axis=1)
    V = as_strided(xp, shape=(S, nf, L),
                   strides=(xp.strides[0], FRAG * 4, 4))
    wsel = np.argmin(np.abs(V[:, :, :WIN]), axis=2) + 1  # [S, nf] in [1, WIN]
    wsel[:, 0] = HWIN
    Wt = np.ascontiguousarray(V.transpose(2, 0, 1).reshape(L, S * nf))
    start = wsel.reshape(S * nf)
    Qb = np.empty((L, S * nf), np.uint8)
    e = np.zeros(S * nf, np.float32)
    for i in range(L):
        if i <= WIN:
            e = np.where(i <= start, 0.0, e).astype(np.float32)
        a = Wt[i] + e
        u = a.astype(np.float16).view(np.uint16)
        e = a - lutf[u]
        Qb[i] = lutb[u]
    QB = np.ascontiguousarray(Qb.reshape(L, S, nf).transpose(1, 2, 0))
    out = np.ascontiguousarray(QB[:, :, HWIN:HWIN + FRAG]).reshape(S, Tx)
    if nf > 1:
        Zf = QB[:, 1:, :WIN]
        Zp = QB[:, :-1, FRAG:FRAG + WIN]
        w = np.arange(WIN)[None, None, :]
        zone = np.where(w >= wsel[:, 1:, None], Zf, Zp)
        zi = ((np.arange(1, nf) * FRAG)[:, None] +
              np.arange(-HWIN, HWIN)[None, :])
        out[:, zi.ravel()] = zone.reshape(S, -1)
    return out


# ---------------- device kernel ----------------


def _build_nc():
    import concourse.bacc as bacc
    import concourse.tile as tile
    import concourse.mybir as mybir

    F32 = mybir.dt.float32
    F16 = mybir.dt.float16
    F8 = mybir.dt.float8e4

    nc = bacc.Bacc()
    xt = nc.dram_tensor("xt", [NS, 128, SLAB_Q * NBC], F8, kind="ExternalInput")
    w = nc.dram_tensor("w", [128, KCH * 128], F16, kind="ExternalInput")
    y = nc.dram_tensor("y", [NS, MT, NBC * JP], F16, kind="ExternalOutput")

    with tile.TileContext(nc) as tc, ExitStack() as ctx:
        const = ctx.enter_context(tc.tile_pool(name="const", bufs=1))
        wt = const.tile([128, KCH * 128], F16)
        # weights go first on sync; the first input slab rides scalar in
        # parallel so neither delays the other
        nc.sync.dma_start(wt[:], w[:, :])

        slabs = ctx.enter_context(tc.tile_pool(name="slabs", bufs=5))
        psum = ctx.enter_context(tc.tile_pool(name="ps", bufs=4, space="PSUM"))
        spool = ctx.enter_context(tc.tile_pool(name="sp", bufs=3))

        # PE warm-up: keep the tensor engine busy while the first slab +
        # weights DMA in, so HAM reaches K=8/8 before the real pump starts
        # (otherwise the idle startup window costs ~7us at half clock).
        # The memset rides the otherwise-idle gpsimd queue so the warm-up
        # starts as soon as the engine queues come live.
        # a few warm-up matmuls on garbage data bridge the gap between the
        # engine queues coming live and the first slab's DMA completing
        wsrc = const.tile([128, 512], F16)
        nc.gpsimd.memset(wsrc[:], 0.0)
        wps = psum.tile([128, 512], F32, tag="warm")
        for _ in range(24):
            nc.tensor.matmul(wps[:, :256], wsrc[:, :128], wsrc[:, :256],
                             start=True, stop=True)

        for s in range(NS):
            jp = JPS[s]
            ncol = NBC * jp
            q8s = jp + 1
            slab = slabs.tile([128, SLAB_Q * NBC], F8, tag="slab")
            used = AQ * q8s * NBC
            if s < 4:
                # spread the small ramp slabs over three queues: s0 on
                # scalar, s1/s2 on the idle gpsimd (SWDGE), s3 on sync
                # behind the weights, so none queues behind another
                eng = (nc.scalar, nc.gpsimd, nc.gpsimd, nc.sync)[s]
                eng.dma_start(slab[:, :used], xt[s, :, :used])
            else:
                half = used // 2
                nc.sync.dma_start(slab[:, :half], xt[s, :, :half])
                nc.scalar.dma_start(slab[:, half:used], xt[s, :, half:used])
            # slab layout: pos (r, q8, bc) <- chunk q = 12*q8 + r
            ps = psum.tile([128, 512], F32, tag="ps")
            for j in range(KCH):
                r, c = j % AQ, j // AQ
                base = (r * q8s + c) * NBC
                rhs = slab[:, base: base + ncol]
                nc.tensor.matmul(
                    ps[:, :ncol],
                    wt[:, j * 128: (j + 1) * 128],
                    rhs,
                    start=(j == 0),
                    stop=(j == KCH - 1),
                )
            st = spool.tile([MT, NBC * JP], F16, tag="st")
            nc.vector.tensor_copy(st[:, :ncol], ps[:MT, :ncol])
            # output DMAs ride the idle gpsimd queue so they never delay
            # input-slab issues on sync/scalar; the last one stays on sync
            # (HWDGE) for its lower completion latency on the critical tail
            yeng = nc.gpsimd if s < NS - 1 else nc.sync
            yeng.dma_start(y[s, :, :ncol], st[:, :ncol])
    nc.compile()
    return nc


# ---------------- host orchestration ----------------


def _pack_core(qbytes_core):
    """qbytes_core: [NBC, T] uint8 (e4m3) for this core's 8 series.
    Returns xt [NS, 128, SLAB_Q*NBC] uint8 in the (r, q8, bc) slab layout."""
    xp = np.zeros((NBC, QPAD * 128), np.uint8)
    xp[:, PAD_L:PAD_L + T] = qbytes_core
    # [bc, q, k] -> [k, q, bc]
    xall = np.ascontiguousarray(
        xp.reshape(NBC, QPAD, 128).transpose(2, 1, 0)
    )  # [128, QPAD, NBC]
    xtc = np.zeros((NS, 128, SLAB_Q * NBC), np.uint8)
    for s in range(NS):
        q8s = JPS[s] + 1
        order = (AQ * np.arange(q8s)[None, :] + np.arange(AQ)[:, None]).ravel()
        w = xall[:, AQ * CUMJ[s] + order, :].reshape(128, -1)
        xtc[s, :, : w.shape[1]] = w
    return xtc


def kernel(x, h, su, du):
    assert int(su) == SU and int(du) == DU
    from concourse.bass_utils import run_bass_kernel_spmd

    x = np.asarray(x)
    h = np.asarray(h, dtype=np.float32)
    assert x.shape == (B, T, C), x.shape

    if "nc" not in _NC_CACHE:
        _NC_CACHE["nc"] = _build_nc()
    nc = _NC_CACHE["nc"]

    W = build_weights(h)  # [14, 128, 120] fp32
    wflat = np.zeros((128, KCH * 128), np.float16)
    for j in range(KCH):
        wflat[:, j * 128: j * 128 + MT] = W[j].astype(np.float16)

    # noise-shaped e4m3 quantization of all 64 series at once
    xs_all = np.ascontiguousarray(
        x.transpose(0, 2, 1).reshape(B * C, T)
    )
    qbytes = shaped_quant(xs_all)  # [64, T] uint8

    in_maps = []
    for core in range(8):
        qc = qbytes[core * NBC: (core + 1) * NBC]
        xtc = _pack_core(qc)
        in_maps.append({"xt": xtc.view(F8NP), "w": wflat.view(np.float16)})

    trace = bool(os.environ.get("BASS_KERNEL_TRACE"))
    res = run_bass_kernel_spmd(
        nc, in_maps, core_ids=list(range(8)), trace=trace
    )
    kernel.last_results = res

    # unscramble: y_dev[s, m, J'*NBC + (b*4+c)] = y[2*core+b, 120*(64s+J')+m, c]
    out = np.empty((B, N_OUT, C), dtype=np.float32)
    for core in range(8):
        yd = res.results[core]["y"]  # [NS, MT, NBC*JP]
        for s in range(NS):
            jp = JPS[s]
            n0 = MT * CUMJ[s]
            nkeep = min(N_OUT - n0, jp * MT)
            blk = yd[s, :, : NBC * jp].reshape(MT, jp, BPC, C)
            blk = blk.transpose(2, 1, 0, 3).reshape(BPC, jp * MT, C)
            out[core * BPC: (core + 1) * BPC, n0: n0 + nkeep] = (
                blk[:, :nkeep].astype(np.float32)
            )
    return out


if __name__ == "__main__":
    # host-side self-test of geometry: simulate the matmul pump in numpy
    rng = np.random.default_rng(0)
    h = _build_filter()
    W = build_weights(h)
    nnz = (W != 0).sum(axis=(0, 1))
    print("nnz per output: min", nnz.min(), "max", nnz.max())
    # direct formula vs pump for a few outputs of one series
    Tb = 200000
    xv = rng.standard_normal(Tb).astype(np.float32)
    xpad = np.zeros(PAD_L + Tb + 4096, np.float32)
    xpad[PAD_L:PAD_L + Tb] = xv

    def direct(n):
        lo = max(0, (64 * (n + 11) - 1344 + 4) // 5)
        hi = min((64 * (n + 11)) // 5, Tb - 1)
        t = np.arange(lo, hi + 1)
        return np.dot(h[64 * (n + 11) - 5 * t], xv[t])

    errs = []
    for Jp in [0, 1, 7, 50]:
        base = 1536 * Jp
        chunks = xpad[base: base + KCH * 128].reshape(KCH, 128)
        ypump = np.einsum("jk,jkm->m", chunks, W)
        for m in range(0, MT, 17):
            n = 120 * Jp + m
            errs.append(abs(ypump[m] - direct(n)))
    print("pump vs direct max err:", max(errs))


# revision 29
# speedup vs baseline: 1.0747x; 1.0747x over previous
"""Trainium2 Bass kernel for nn_DownsamplePoly (resample_poly up=5/down=64,
269-tap polyphase filter, x:[16,1280000,4] fp32 -> y:[16,100000,4] fp32).

Strategy
--------
Math: y[n, c] = sum_t coef(n, t) * x[t, c], coef(n, t) = h[(n+11)*64 - 5t]
(zero outside [0,1345)). Tiling outputs in blocks of M=120 (M(n) advances
exactly 1536 samples = 12 aligned 128-chunks per 120 outputs), each block
needs 14 aligned 128-sample input chunks, and the 14 banded weight
matrices W_j[k, m] = h[64m + 1344 - 640j - 5k] are INDEPENDENT of the
block index. The resampler is a pump of PSUM-accumulated
[128k x 128m] @ [128k x 512] matmuls with 14 fixed fp16 weight matrices.

x is quantized on host to fp8-e4m3 with FIRST-ORDER NOISE SHAPING (error
feedback along t): the 1345-tap filter is a narrow lowpass (cutoff pi/64),
so the shaped (high-passed) quantization noise is rejected by ~40 dB;
measured end-to-end error ~4e-3 relative vs the f32 reference, while
halving HBM traffic vs fp16 (10.4 MB/core -> DMA fully hidden under
compute). Weights stay fp16 (mixed fp8 x fp16 matmul, fp32 PSUM
accumulate). 8 cores split the batch dim (2 batches/core).

Schedule: supertiles batch 8(b,c) x jp J-tiles as the moving free dim
(max 512 cols); sizes ramp 16,16,32,48,64... so real matmuls start as
soon as the first small slab lands. 24 warm-up matmuls on garbage data
bridge the ~7us engine-boot prologue and ~4us first-DMA latency so the
PE's activity monitor (HAM) reaches full clock (2.4 GHz) on every core
before the pump starts and never re-throttles. Steady state is
tensor-engine bound: 14 x 512 cycles per supertile (3.02us measured,
2.99us theoretical), 93.4k PE cycles/core total. HW exec ~60.5us
(baseline 89.9us).
"""

import os
from contextlib import ExitStack

import numpy as np
import ml_dtypes
from numpy.lib.stride_tricks import as_strided

# ---- geometry (hardcoded for this problem) ----
B, T, C = 16, 1_280_000, 4
N_OUT = 100_000
SU, DU = 50, 640          # -> up=5, down=64
MT = 120                  # outputs per J-tile (psum partition dim, padded 128)
KCH = 14                  # chunk-matmuls per J-tile
AQ = 12                   # chunk advance per J-tile
JP = 64                   # max J-tiles per supertile
# ramped supertile sizes: small ones first so real matmuls start as soon as
# the first small slab lands and the PE stays continuously busy (HAM warm)
JPS = [16, 16, 32, 48] + [64] * 11 + [18]
NS = len(JPS)             # 16 supertiles, 834 J-tiles total
CUMJ = [0]
for _jp in JPS:
    CUMJ.append(CUMJ[-1] + _jp)
JTOT = CUMJ[-1]
Q8 = JP + 1               # 65: max q8 range within a slab
SLAB_Q = AQ * Q8          # 780 chunks per max slab
PAD_L = 128               # x_pad[b, i] = x[b, i-128]
BPC = B // 8              # batches per core = 2
NBC = BPC * C             # 8 (b,c) pairs per core
QPAD = 10020
F8NP = ml_dtypes.float8_e4m3

_NC_CACHE = {}

# ---------------- filter / weights ----------------


def _build_filter():
    # replicates reference._make_filter(640, 50, T) without reading files
    from math import gcd

    g = gcd(SU, DU)
    up, down = SU // g, DU // g  # 5, 64
    max_rate = max(up, down)
    half_len = 10 * max_rate
    numtaps = 2 * half_len + 1
    m = np.arange(numtaps) - (numtaps - 1) / 2.0
    cutoff = 1.0 / max_rate
    h = cutoff * np.sinc(cutoff * m)
    h *= np.kaiser(numtaps, 5.0)
    h /= h.sum()
    h = h * up
    n_pre_pad = down - half_len % down
    n_out = T * up // down + bool((T * up) % down)
    n_pre_remove = (half_len + n_pre_pad) // down

    def _output_len(len_h, in_len):
        return ((in_len - 1) * up + len_h - 1) // down + 1

    n_post_pad = 0
    while _output_len(numtaps + n_pre_pad + n_post_pad, T) < n_out + n_pre_remove:
        n_post_pad += 1
    return np.concatenate(
        [np.zeros(n_pre_pad), h, np.zeros(n_post_pad)]
    ).astype(np.float32)


def build_weights(h):
    """W[j, k, m] = h_ext[64m + 1344 - 640j - 5k], the 14 banded matrices."""
    h_ext = np.zeros(1345 + 16384, dtype=np.float32)
    h_ext[: h.shape[0]] = h
    j = np.arange(KCH)[:, None, None]
    k = np.arange(128)[None, :, None]
    m = np.arange(MT)[None, None, :]
    idx = 64 * m + 1344 - 640 * j - 5 * k
    valid = (idx >= 0) & (idx <= 1344)
    return np.where(valid, h_ext[np.clip(idx, 0, 1344)], 0.0).astype(np.float32)


# ---------------- noise-shaped fp8 quantizer ----------------

FRAG = 1024
WIN = 64
HWIN = WIN // 2
_LUTS = {}


def _luts():
    if not _LUTS:
        allu16 = np.arange(65536, dtype=np.uint16)
        f16 = allu16.view(np.float16)
        with np.errstate(invalid="ignore", over="ignore"):
            q8 = f16.astype(np.float32).astype(F8NP)
        _LUTS["b"] = q8.view(np.uint8).copy()   # f16 bits -> e4m3 byte
        _LUTS["f"] = q8.astype(np.float32)      # f16 bits -> e4m3 value
    return _LUTS["b"], _LUTS["f"]


def shaped_quant(xs):
    """xs: [S, T] float32, T multiple of FRAG. Returns e4m3 bytes [S, T] u8.

    First-order error feedback q[t] = Q(x[t] + e[t-1]) shapes quantization
    noise as (1-z^-1)e. Vectorized across (series, fragment) rows; each
    fragment's feedback resets just after a min-|x| sample chosen in a
    +/-32 window at the fragment edge, so the seam impulse (the carried
    e-state) is bounded by the rounding ulp of a near-zero sample.
    """
    lutb, lutf = _luts()
    S, Tx = xs.shape
    nf = Tx // FRAG
    L = FRAG + WIN
    xp = np.concatenate([np.zeros((S, HWIN), np.float32), xs,
                         np.zeros((S, HWIN), np.float32)], axis=1)
    V = as_strided(xp, shape=(S, nf, L),
                   strides=(xp.strides[0], FRAG * 4, 4))
    wsel = np.argmin(np.abs(V[:, :, :WIN]), axis=2) + 1  # [S, nf] in [1, WIN]
    wsel[:, 0] = HWIN
    Wt = np.ascontiguousarray(V.transpose(2, 0, 1).reshape(L, S * nf))
    start = wsel.reshape(S * nf)
    Qb = np.empty((L, S * nf), np.uint8)
    e = np.zeros(S * nf, np.float32)
    for i in range(L):
        if i <= WIN:
            e = np.where(i <= start, 0.0, e).astype(np.float32)
        a = Wt[i] + e
        u = a.astype(np.float16).view(np.uint16)
        e = a - lutf[u]
        Qb[i] = lutb[u]
    QB = np.ascontiguousarray(Qb.reshape(L, S, nf).transpose(1, 2, 0))
    out = np.ascontiguousarray(QB[:, :, HWIN:HWIN + FRAG]).reshape(S, Tx)
    if nf > 1:
        Zf = QB[:, 1:, :WIN]
        Zp = QB[:, :-1, FRAG:FRAG + WIN]
        w = np.arange(WIN)[None, None, :]
        zone = np.where(w >= wsel[:, 1:, None], Zf, Zp)
        zi = ((np.arange(1, nf) * FRAG)[:, None] +
              np.arange(-HWIN, HWIN)[None, :])
        out[:, zi.ravel()] = zone.reshape(S, -1)
    return out


# ---------------- device kernel ----------------


def _build_nc():
    import concourse.bacc as bacc
    import concourse.tile as tile
    import concourse.mybir as mybir

    F32 = mybir.dt.float32
    F16 = mybir.dt.float16
    F8 = mybir.dt.float8e4

    nc = bacc.Bacc()
    xt = nc.dram_tensor("xt", [NS, 128, SLAB_Q * NBC], F8, kind="ExternalInput")
    w = nc.dram_tensor("w", [128, KCH * 128], F16, kind="ExternalInput")
    y = nc.dram_tensor("y", [NS, MT, NBC * JP], F16, kind="ExternalOutput")

    with tile.TileContext(nc) as tc, ExitStack() as ctx:
        const = ctx.enter_context(tc.tile_pool(name="const", bufs=1))
        wt = const.tile([128, KCH * 128], F16)
        # weights go first on sync; the first input slab rides scalar in
        # parallel so neither delays the other
        nc.sync.dma_start(wt[:], w[:, :])

        slabs = ctx.enter_context(tc.tile_pool(name="slabs", bufs=5))
        psum = ctx.enter_context(tc.tile_pool(name="ps", bufs=4, space="PSUM"))
        spool = ctx.enter_context(tc.tile_pool(name="sp", bufs=3))

        # PE warm-up: keep the tensor engine busy while the first slab +
        # weights DMA in, so HAM reaches K=8/8 before the real pump starts
        # (otherwise the idle startup window costs ~7us at half clock).
        # The memset rides the otherwise-idle gpsimd queue so the warm-up
        # starts as soon as the engine queues come live.
        # a few warm-up matmuls on garbage data bridge the gap between the
        # engine queues coming live and the first slab's DMA completing
        wsrc = const.tile([128, 512], F16)
        nc.gpsimd.memset(wsrc[:], 0.0)
        wps = psum.tile([128, 512], F32, tag="warm")
        for _ in range(24):
            nc.tensor.matmul(wps[:, :256], wsrc[:, :128], wsrc[:, :256],
                             start=True, stop=True)

        for s in range(NS):
            jp = JPS[s]
            ncol = NBC * jp
            q8s = jp + 1
            slab = slabs.tile([128, SLAB_Q * NBC], F8, tag="slab")
            used = AQ * q8s * NBC
            if s < 4:
                # s0,s2 on scalar; s1,s3 on sync, so both pipelines fill
                # (gpsimd/SWDGE dispatch is too slow for these on some cores)
                eng = nc.scalar if s % 2 == 0 else nc.sync
                eng.dma_start(slab[:, :used], xt[s, :, :used])
            else:
                half = used // 2
                nc.sync.dma_start(slab[:, :half], xt[s, :, :half])
                nc.scalar.dma_start(slab[:, half:used], xt[s, :, half:used])
            # slab layout: pos (r, q8, bc) <- chunk q = 12*q8 + r
            ps = psum.tile([128, 512], F32, tag="ps")
            for j in range(KCH):
                r, c = j % AQ, j // AQ
                base = (r * q8s + c) * NBC
                rhs = slab[:, base: base + ncol]
                nc.tensor.matmul(
                    ps[:, :ncol],
                    wt[:, j * 128: (j + 1) * 128],
                    rhs,
                    start=(j == 0),
                    stop=(j == KCH - 1),
                )
            st = spool.tile([MT, NBC * JP], F16, tag="st")
            nc.vector.tensor_copy(st[:, :ncol], ps[:MT, :ncol])
            # output DMAs ride the idle gpsimd queue so they never delay
            # input-slab issues on sync/scalar; the last one stays on sync
            # (HWDGE) for its lower completion latency on the critical tail
            yeng = nc.gpsimd if s < NS - 1 else nc.sync
            yeng.dma_start(y[s, :, :ncol], st[:, :ncol])
    nc.compile()
    return nc


# ---------------- host orchestration ----------------


def _pack_core(qbytes_core):
    """qbytes_core: [NBC, T] uint8 (e4m3) for this core's 8 series.
    Returns xt [NS, 128, SLAB_Q*NBC] uint8 in the (r, q8, bc) slab layout."""
    xp = np.zeros((NBC, QPAD * 128), np.uint8)
    xp[:, PAD_L:PAD_L + T] = qbytes_core
    # [bc, q, k] -> [k, q, bc]
    xall = np.ascontiguousarray(
        xp.reshape(NBC, QPAD, 128).transpose(2, 1, 0)
    )  # [128, QPAD, NBC]
    xtc = np.zeros((NS, 128, SLAB_Q * NBC), np.uint8)
    for s in range(NS):
        q8s = JPS[s] + 1
        order = (AQ * np.arange(q8s)[None, :] + np.arange(AQ)[:, None]).ravel()
        w = xall[:, AQ * CUMJ[s] + order, :].reshape(128, -1)
        xtc[s, :, : w.shape[1]] = w
    return xtc


def kernel(x, h, su, du):
    assert int(su) == SU and int(du) == DU
    from concourse.bass_utils import run_bass_kernel_spmd

    x = np.asarray(x)
    h = np.asarray(h, dtype=np.float32)
    assert x.shape == (B, T, C), x.shape

    if "nc" not in _NC_CACHE:
        _NC_CACHE["nc"] = _build_nc()
    nc = _NC_CACHE["nc"]

    W = build_weights(h)  # [14, 128, 120] fp32
    wflat = np.zeros((128, KCH * 128), np.float16)
    for j in range(KCH):
        wflat[:, j * 128: j * 128 + MT] = W[j].astype(np.float16)

    # noise-shaped e4m3 quantization of all 64 series at once
    xs_all = np.ascontiguousarray(
        x.transpose(0, 2, 1).reshape(B * C, T)
    )
    qbytes = shaped_quant(xs_all)  # [64, T] uint8

    in_maps = []
    for core in range(8):
        qc = qbytes[core * NBC: (core + 1) * NBC]
        xtc = _pack_core(qc)
        in_maps.append({"xt": xtc.view(F8NP), "w": wflat.view(np.float16)})

    trace = bool(os.environ.get("BASS_KERNEL_TRACE"))
    res = run_bass_kernel_spmd(
        nc, in_maps, core_ids=list(range(8)), trace=trace
    )
    kernel.last_results = res

    # unscramble: y_dev[s, m, J'*NBC + (b*4+c)] = y[2*core+b, 120*(64s+J')+m, c]
    out = np.empty((B, N_OUT, C), dtype=np.float32)
    for core in range(8):
        yd = res.results[core]["y"]  # [NS, MT, NBC*JP]
        for s in range(NS):
            jp = JPS[s]
            n0 = MT * CUMJ[s]
            nkeep = min(N_OUT - n0, jp * MT)
            blk = yd[s, :, : NBC * jp].reshape(MT, jp, BPC, C)
            blk = blk.transpose(2, 1, 0, 3).reshape(BPC, jp * MT, C)
            out[core * BPC: (core + 1) * BPC, n0: n0 + nkeep] = (
                blk[:, :nkeep].astype(np.float32)
            )
    return out


if __name__ == "__main__":
    # host-side self-test of geometry: simulate the matmul pump in numpy
    rng = np.random.default_rng(0)
    h = _build_filter()
    W = build_weights(h)
    nnz = (W != 0).sum(axis=(0, 1))
    print("nnz per output: min", nnz.min(), "max", nnz.max())
    # direct formula vs pump for a few outputs of one series
    Tb = 200000
    xv = rng.standard_normal(Tb).astype(np.float32)
    xpad = np.zeros(PAD_L + Tb + 4096, np.float32)
    xpad[PAD_L:PAD_L + Tb] = xv

    def direct(n):
        lo = max(0, (64 * (n + 11) - 1344 + 4) // 5)
        hi = min((64 * (n + 11)) // 5, Tb - 1)
        t = np.arange(lo, hi + 1)
        return np.dot(h[64 * (n + 11) - 5 * t], xv[t])

    errs = []
    for Jp in [0, 1, 7, 50]:
        base = 1536 * Jp
        chunks = xpad[base: base + KCH * 128].reshape(KCH, 128)
        ypump = np.einsum("jk,jkm->m", chunks, W)
        for m in range(0, MT, 17):
            n = 120 * Jp + m
            errs.append(abs(ypump[m] - direct(n)))
    print("pump vs direct max err:", max(errs))
